# revision 6
# baseline (speedup 1.0000x reference)
"""Epipolar cross-attention Trainium2 kernel.

Full inputs -> shard over 8 cores as (batch b = core//2, query-half = core%2),
each core computes 2048 query rows against all M=4096 keys of its batch.

Math (matches reference exactly):
  Q = nodes_L @ Wq^T + bq ; K = nodes_R @ Wk^T + bk ; V = nodes_R @ Wv^T + bv
  mask = (|vL-vR| < 3) & (uL-uR > 0) & (uL-uR < 192)
  S = where(mask, QK^T/sqrt(C), -1e9); W = softmax(S)
  out = (W@V) @ Wm^T + bm ; disp = sum(W*(uL-uR)) ; conf = any(mask)

Device trick: penalty form. q = mask*512 (exact fp32 compares on DVE),
PSUM += identity @ q (bf16), then E = exp(S + 512*mask - 512):
  masked -> exp(s-512) == 0 exactly (fp32 underflow)  [matches where(): 0]
  valid  -> exp(s) (logit quantized at ulp(512)=6.1e-5)
Empty rows: E == 0 -> Z == 0 -> W = 1/4096 uniform via fused scalar add.
W @ [V, uR, 1] aug-matmul gives matched, sum(W*uR), sum(W) in one pass;
disp = uL*sumW - sum(W*uR) handles uniform rows automatically.
"""
import os
import numpy as np

B, N, M, C = 4, 4096, 4096, 256
NSH = N // 2          # rows per core
P = 128
NB = NSH // P         # 16 n-blocks
MC = 1024             # m-chunk
NMC = M // MC         # 4 chunks
PEN = 512.0           # penalty scale (exact in bf16; exp(s-512) == 0)

_CACHE = {}
LAST_RESULT = None


def _build():
    import concourse.bacc as bacc
    import concourse.tile as tile
    import concourse.mybir as mybir
    from concourse.masks import make_identity

    f32 = mybir.dt.float32
    f32r = mybir.dt.float32r
    bf16 = mybir.dt.bfloat16
    A = mybir.AluOpType
    AF = mybir.ActivationFunctionType

    nc = bacc.Bacc(None)

    nl_d = nc.dram_tensor("nodesL", [NSH, C], f32, kind="ExternalInput")
    nr_d = nc.dram_tensor("nodesR", [M, C], f32, kind="ExternalInput")
    uL_d = nc.dram_tensor("uL", [NSH], f32, kind="ExternalInput")
    vL_d = nc.dram_tensor("vL", [NSH], f32, kind="ExternalInput")
    uR_d = nc.dram_tensor("uR", [1, M], f32, kind="ExternalInput")
    vR_d = nc.dram_tensor("vR", [1, M], f32, kind="ExternalInput")
    wq_d = nc.dram_tensor("Wq", [C, C], f32, kind="ExternalInput")
    wk_d = nc.dram_tensor("Wk", [C, C], f32, kind="ExternalInput")
    wv_d = nc.dram_tensor("Wv", [C, C], f32, kind="ExternalInput")
    wm_d = nc.dram_tensor("Wm", [C, C], f32, kind="ExternalInput")
    bq_d = nc.dram_tensor("bq", [1, C], f32, kind="ExternalInput")
    bk_d = nc.dram_tensor("bk", [1, C], f32, kind="ExternalInput")
    bv_d = nc.dram_tensor("bv", [1, C], f32, kind="ExternalInput")
    bm_d = nc.dram_tensor("bm", [1, C], f32, kind="ExternalInput")

    attn_d = nc.dram_tensor("attn", [NSH, M], f32, kind="ExternalOutput")
    out_d = nc.dram_tensor("out", [NSH, C], f32, kind="ExternalOutput")
    disp_d = nc.dram_tensor("disp", [NSH], f32, kind="ExternalOutput")
    conf_d = nc.dram_tensor("conf", [NSH], f32, kind="ExternalOutput")

    KC = C // P  # 2 contraction chunks
    attn_v = attn_d.rearrange("(nb p) m -> nb p m", p=P)
    out_v = out_d.rearrange("(nb p) c -> nb p c", p=P)

    with tile.TileContext(nc) as tc:
        with tc.tile_pool(name="const", bufs=1) as cst:
            # ---------------- resident tensors ----------------
            qT = cst.tile([P, KC, NSH], f32r, tag="qT")        # [d, n]
            kT = cst.tile([P, KC, M], f32r, tag="kT")          # [d, m]
            vaug = cst.tile([P, M // P, C + 2], f32r, tag="vaug")  # [m, d|uR|1]
            uRb = cst.tile([P, M], f32, tag="uRb")
            vRb = cst.tile([P, M], f32, tag="vRb")
            uL_c = cst.tile([P, NB], f32, tag="uL_c")
            vL_c = cst.tile([P, NB], f32, tag="vL_c")
            idb = cst.tile([P, P], bf16, tag="idb")            # identity bf16
            idr = cst.tile([P, P], f32r, tag="idr")            # identity f32r
            ones512 = cst.tile([1, 512], f32r, tag="ones512")
            onesP = cst.tile([1, P], f32r, tag="onesP")
            bqr = cst.tile([1, C], f32r, tag="bqr")            # bq/16
            bkr = cst.tile([1, C], f32r, tag="bkr")
            bvr = cst.tile([1, C], f32r, tag="bvr")
            bmr = cst.tile([1, C], f32r, tag="bmr")
            wqT = cst.tile([P, KC, C], f32r, tag="wqT")        # [c_in, d]/16
            wkT = cst.tile([P, KC, C], f32r, tag="wkT")
            wvT = cst.tile([P, KC, C], f32r, tag="wvT")
            wmT = cst.tile([P, KC, C], f32r, tag="wmT")        # [d, c]
            nb512 = cst.tile([P, 1], f32, tag="nb512")         # -512 bias
            disp_all = cst.tile([P, NB], f32, tag="disp_all")
            conf_all = cst.tile([P, NB], f32, tag="conf_all")

            # ---------------- loads ----------------
            nc.sync.dma_start(uL_c[:], uL_d.rearrange("(nb p) -> p nb", p=P))
            nc.sync.dma_start(vL_c[:], vL_d.rearrange("(nb p) -> p nb", p=P))
            nc.sync.dma_start(uRb[:], uR_d[:].partition_broadcast(P))
            nc.sync.dma_start(vRb[:], vR_d[:].partition_broadcast(P))
            nc.gpsimd.dma_start(vaug[:, :, C],
                                uR_d.rearrange("one (a p) -> (one p) a", p=P))

            make_identity(nc, idb[:])
            idf = cst.tile([P, P], f32, tag="idf")
            make_identity(nc, idf[:])
            nc.scalar.copy(idr[:], idf[:])
            zsrc = cst.tile([1, 512], f32, tag="zsrc")
            nc.gpsimd.memset(zsrc[:], 0.0)
            nc.scalar.activation(ones512[:], zsrc[:], AF.Copy, bias=1.0, scale=0.0)
            nc.scalar.activation(onesP[:], zsrc[:, :P], AF.Copy, bias=1.0, scale=0.0)
            # vaug ones column: value-independent copy-const from uRb slice
            nc.scalar.activation(vaug[:, :, C + 1], uRb[:, : M // P],
                                 AF.Copy, bias=1.0, scale=0.0)
            nc.gpsimd.memset(nb512[:], -PEN)

            btmp = cst.tile([1, C], f32, tag="btmp")
            for b_dram, b_sb, scl in ((bq_d, bqr, 0.0625), (bk_d, bkr, 1.0),
                                      (bv_d, bvr, 1.0), (bm_d, bmr, 1.0)):
                nc.sync.dma_start(btmp[:], b_dram[:])
                nc.scalar.activation(b_sb[:], btmp[:], AF.Copy, bias=0.0, scale=scl)

            # ---------------- prologue: transposes + projections ----------------
            with (
                tc.tile_pool(name="prol", bufs=2) as prol,
                tc.tile_pool(name="ppsum", bufs=2, space="PSUM") as pps,
            ):
                # weights: natural [row-part, col] -> transposed [col-part, row]
                for w_dram, w_out, scl in (
                    (wq_d, wqT, 0.0625), (wk_d, wkT, 1.0), (wv_d, wvT, 1.0),
                    (wm_d, wmT, 1.0),
                ):
                    wnat = prol.tile([P, KC, C], f32r, tag="wnat")
                    nc.gpsimd.dma_start(wnat[:],
                                        w_dram.rearrange("(a p) c -> p a c", p=P))
                    for a in range(KC):
                        for bb in range(KC):
                            tp = pps.tile([P, P], f32r, tag="tp_w")
                            nc.tensor.transpose(tp[:], wnat[:, a, bb * P:(bb + 1) * P],
                                                idr[:])
                            nc.scalar.activation(w_out[:, bb, a * P:(a + 1) * P],
                                                 tp[:].bitcast(f32), AF.Copy,
                                                 bias=0.0, scale=scl)

                # nodes_R^T halves; K^T[d,m] (+bk); V[m,d] (+bv) -> vaug
                mh = M // 2
                for half in range(2):
                    nrT = prol.tile([P, KC, mh], f32r, tag="nrT")
                    nrnat = prol.tile([P, mh // P, C], f32r, tag="nrnat")
                    nc.gpsimd.dma_start(
                        nrnat[:],
                        nr_d[half * mh:(half + 1) * mh].rearrange("(a p) c -> p a c", p=P))
                    for a2 in range(mh // (2 * P)):
                        tpn = pps.tile([P, KC, 2 * P], f32r, tag="tp_n")
                        for a1 in range(2):
                            a = a2 * 2 + a1
                            for bb in range(KC):
                                nc.tensor.transpose(tpn[:, bb, a1 * P:(a1 + 1) * P],
                                                    nrnat[:, a, bb * P:(bb + 1) * P],
                                                    idr[:])
                        nc.scalar.copy(nrT[:, :, a2 * 2 * P:(a2 + 1) * 2 * P],
                                       tpn[:].bitcast(f32))
                    for dd in range(KC):
                        for mt in range(mh // 512):
                            ps_k = pps.tile([P, 512], f32, tag="ps_k")
                            for k in range(KC):
                                nc.tensor.matmul(ps_k[:], wkT[:, k, dd * P:(dd + 1) * P],
                                                 nrT[:, k, mt * 512:(mt + 1) * 512],
                                                 start=(k == 0), stop=False)
                            nc.tensor.matmul(ps_k[:], bkr[:, dd * P:(dd + 1) * P],
                                             ones512[:], start=False, stop=True)
                            lo = half * mh + mt * 512
                            nc.scalar.copy(kT[:, dd, lo:lo + 512], ps_k[:])
                    for a in range(mh // P):
                        ps_v = pps.tile([P, C], f32, tag="ps_v")
                        for k in range(KC):
                            nc.tensor.matmul(ps_v[:], nrT[:, k, a * P:(a + 1) * P],
                                             wvT[:, k, :], start=(k == 0), stop=False)
                        nc.tensor.matmul(ps_v[:], onesP[:], bvr[:],
                                         start=False, stop=True)
                        nc.scalar.copy(vaug[:, half * (mh // P) + a, :C], ps_v[:])

                # nodes_L^T halves; Q^T[d,n] = (Wq/16)@nlT + bq/16
                nh = NSH // 2
                for half in range(2):
                    nlT = prol.tile([P, KC, nh], f32r, tag="nrT")
                    nlnat = prol.tile([P, nh // P, C], f32r, tag="nrnat")
                    nc.gpsimd.dma_start(
                        nlnat[:],
                        nl_d[half * nh:(half + 1) * nh].rearrange("(a p) c -> p a c", p=P))
                    for a2 in range(nh // (2 * P)):
                        tpn = pps.tile([P, KC, 2 * P], f32r, tag="tp_n")
                        for a1 in range(2):
                            a = a2 * 2 + a1
                            for bb in range(KC):
                                nc.tensor.transpose(tpn[:, bb, a1 * P:(a1 + 1) * P],
                                                    nlnat[:, a, bb * P:(bb + 1) * P],
                                                    idr[:])
                        nc.scalar.copy(nlT[:, :, a2 * 2 * P:(a2 + 1) * 2 * P],
                                       tpn[:].bitcast(f32))
                    for dd in range(KC):
                        for nt in range(nh // 512):
                            ps_q = pps.tile([P, 512], f32, tag="ps_k")
                            for k in range(KC):
                                nc.tensor.matmul(ps_q[:], wqT[:, k, dd * P:(dd + 1) * P],
                                                 nlT[:, k, nt * 512:(nt + 1) * 512],
                                                 start=(k == 0), stop=False)
                            nc.tensor.matmul(ps_q[:], bqr[:, dd * P:(dd + 1) * P],
                                             ones512[:], start=False, stop=True)
                            lo = half * nh + nt * 512
                            nc.scalar.copy(qT[:, dd, lo:lo + 512], ps_q[:])

            # ---------------- main loop ----------------
            with (
                tc.tile_pool(name="mn", bufs=2) as mn,
                tc.tile_pool(name="epool", bufs=5) as epool,
                tc.tile_pool(name="wpool", bufs=3) as wpool,
                tc.tile_pool(name="small", bufs=2) as sml,
                tc.tile_pool(name="wtp", bufs=4) as wtp,
                tc.tile_pool(name="qk_ps", bufs=2, space="PSUM") as qk_ps,
                tc.tile_pool(name="tp_ps", bufs=2, space="PSUM") as tp_ps,
                tc.tile_pool(name="mt_ps", bufs=1, space="PSUM") as mt_ps,
                tc.tile_pool(name="op_ps", bufs=1, space="PSUM") as op_ps,
            ):
                for i in range(NB):
                    uL_i = uL_c[:, i:i + 1]
                    vL_i = vL_c[:, i:i + 1]
                    z4 = sml.tile([P, NMC], f32, tag="z4")
                    e_ch = []
                    for h in range(NMC):
                        ms = h * MC
                        dv = mn.tile([P, MC], f32, tag="dv")
                        nc.scalar.activation(dv[:], vRb[:, ms:ms + MC], AF.Abs,
                                             bias=vL_i, scale=-1.0)
                        cv = mn.tile([P, MC], bf16, tag="cv")
                        nc.vector.tensor_scalar(cv[:], dv[:], 3.0, PEN,
                                                A.is_lt, A.mult)
                        cu2 = mn.tile([P, MC], bf16, tag="cu2")
                        nc.vector.tensor_scalar(cu2[:], uRb[:, ms:ms + MC], uL_i,
                                                -192.0, A.subtract, A.is_gt)
                        t1 = mn.tile([P, MC], bf16, tag="t1")
                        nc.vector.scalar_tensor_tensor(t1[:], uRb[:, ms:ms + MC],
                                                       uL_i, cv[:], A.is_lt, A.mult)
                        qm = mn.tile([P, MC], bf16, tag="qm")
                        nc.vector.tensor_tensor(qm[:], t1[:], cu2[:], A.mult)
                        ps = qk_ps.tile([P, MC], f32, tag="ps_qk")
                        for t in range(MC // 512):
                            sl = slice(t * 512, (t + 1) * 512)
                            for k in range(KC):
                                nc.tensor.matmul(ps[:, sl],
                                                 qT[:, k, i * P:(i + 1) * P],
                                                 kT[:, k, ms + t * 512:ms + (t + 1) * 512],
                                                 start=(k == 0), stop=False)
                            nc.tensor.matmul(ps[:, sl], idb[:], qm[:, sl],
                                             start=False, stop=True)
                        e = epool.tile([P, MC], f32, tag="e")
                        nc.scalar.activation(e[:], ps[:], AF.Exp, bias=nb512[:],
                                             scale=1.0, accum_out=z4[:, h:h + 1])
                        e_ch.append(e)

                    z = sml.tile([P, 1], f32, tag="z")
                    nc.vector.tensor_reduce(z[:], z4[:], mybir.AxisListType.X, A.add)
                    zadj = sml.tile([P, 1], f32, tag="zadj")
                    nc.vector.scalar_tensor_tensor(zadj[:], z[:], 0.0, z[:],
                                                   A.is_equal, A.add)
                    rz = sml.tile([P, 1], f32, tag="rz")
                    nc.vector.reciprocal(rz[:], zadj[:])
                    ucol = sml.tile([P, 1], f32, tag="ucol")
                    nc.vector.tensor_scalar(ucol[:], z[:], 0.0, 1.0 / M,
                                            A.is_equal, A.mult)
                    nc.vector.tensor_scalar(conf_all[:, i:i + 1], z[:], 0.0, None,
                                            A.is_gt)

                    mt_acc = mt_ps.tile([P, C + 2], f32, tag="mt_acc")
                    for h in range(NMC):
                        w = wpool.tile([P, MC], f32r, tag="w")
                        nc.vector.tensor_scalar(w[:], e_ch[h][:], rz[:], ucol[:],
                                                A.mult, A.add)
                        nc.sync.dma_start(attn_v[i, :, h * MC:(h + 1) * MC],
                                          w[:].bitcast(f32))
                        # 4 transposes per PSUM group, one batched copy, 2 groups
                        for g in range(2):
                            tp = tp_ps.tile([P, MC // 2], f32r, tag="tp_main")
                            for j in range(MC // (2 * P)):
                                jj = g * (MC // (2 * P)) + j
                                nc.tensor.transpose(tp[:, j * P:(j + 1) * P],
                                                    w[:, jj * P:(jj + 1) * P], idr[:])
                            wT = wtp.tile([P, MC // 2], f32r, tag="wT")
                            if (2 * h + g) % 3 == 2:
                                nc.vector.tensor_copy(wT[:], tp[:].bitcast(f32))
                            else:
                                nc.scalar.copy(wT[:], tp[:].bitcast(f32))
                            for j in range(MC // (2 * P)):
                                jg = h * (MC // P) + g * (MC // (2 * P)) + j
                                nc.tensor.matmul(mt_acc[:], wT[:, j * P:(j + 1) * P],
                                                 vaug[:, jg, :],
                                                 start=(jg == 0),
                                                 stop=(jg == M // P - 1))

                    mt_sb = sml.tile([P, C + 2], f32r, tag="mt_sb")
                    nc.scalar.copy(mt_sb[:], mt_acc[:])
                    nc.vector.scalar_tensor_tensor(
                        disp_all[:, i:i + 1], mt_sb[:, C + 1:C + 2].bitcast(f32),
                        uL_i, mt_sb[:, C:C + 1].bitcast(f32), A.mult, A.subtract)

                    mTt = sml.tile([P, KC, P], f32r, tag="mTt")
                    tpm = tp_ps.tile([P, MC // 2], f32r, tag="tp_main")
                    for k in range(KC):
                        nc.tensor.transpose(tpm[:, k * P:(k + 1) * P],
                                            mt_sb[:, k * P:(k + 1) * P], idr[:])
                    nc.scalar.copy(mTt[:], tpm[:, :KC * P].bitcast(f32))
                    op = op_ps.tile([P, C], f32, tag="op")
                    for k in range(KC):
                        nc.tensor.matmul(op[:], mTt[:, k, :], wmT[:, k, :],
                                         start=(k == 0), stop=False)
                    nc.tensor.matmul(op[:], onesP[:], bmr[:], start=False, stop=True)
                    out_sb = sml.tile([P, C], f32, tag="out_sb")
                    nc.scalar.copy(out_sb[:], op[:])
                    nc.sync.dma_start(out_v[i], out_sb[:])

                nc.sync.dma_start(disp_d.rearrange("(nb p) -> p nb", p=P), disp_all[:])
                nc.sync.dma_start(conf_d.rearrange("(nb p) -> p nb", p=P), conf_all[:])

    nc.compile()
    return nc


def kernel(nodes_L, nodes_R, kpts_L, kpts_R, Wq, bq, Wk, bk, Wv, bv, Wm, bm):
    global LAST_RESULT
    from concourse.bass_utils import run_bass_kernel_spmd

    if "nc" not in _CACHE:
        _CACHE["nc"] = _build()
    nc = _CACHE["nc"]

    nodes_L = np.ascontiguousarray(np.asarray(nodes_L, dtype=np.float32))
    nodes_R = np.ascontiguousarray(np.asarray(nodes_R, dtype=np.float32))
    kpts_L = np.asarray(kpts_L, dtype=np.float32)
    kpts_R = np.asarray(kpts_R, dtype=np.float32)
    common = {
        "Wq": np.ascontiguousarray(np.asarray(Wq, np.float32)),
        "Wk": np.ascontiguousarray(np.asarray(Wk, np.float32)),
        "Wv": np.ascontiguousarray(np.asarray(Wv, np.float32)),
        "Wm": np.ascontiguousarray(np.asarray(Wm, np.float32)),
        "bq": np.ascontiguousarray(np.asarray(bq, np.float32)).reshape(1, C),
        "bk": np.ascontiguousarray(np.asarray(bk, np.float32)).reshape(1, C),
        "bv": np.ascontiguousarray(np.asarray(bv, np.float32)).reshape(1, C),
        "bm": np.ascontiguousarray(np.asarray(bm, np.float32)).reshape(1, C),
    }
    in_maps = []
    for core in range(8):
        b, h = core // 2, core % 2
        sl = slice(h * NSH, (h + 1) * NSH)
        in_maps.append({
            "nodesL": np.ascontiguousarray(nodes_L[b, sl]),
            "nodesR": np.ascontiguousarray(nodes_R[b]),
            "uL": np.ascontiguousarray(kpts_L[b, sl, 0]),
            "vL": np.ascontiguousarray(kpts_L[b, sl, 1]),
            "uR": np.ascontiguousarray(kpts_R[b, :, 0]).reshape(1, M),
            "vR": np.ascontiguousarray(kpts_R[b, :, 1]).reshape(1, M),
            **common,
        })

    LAST_RESULT = run_bass_kernel_spmd(nc, in_maps, core_ids=list(range(8)))
    res = LAST_RESULT.results

    out = np.empty((B, N, C), np.float32)
    disp = np.empty((B, N, 1), np.float32)
    conf = np.empty((B, N, 1), np.float32)
    attn = np.empty((B, N, M), np.float32)
    for core in range(8):
        b, h = core // 2, core % 2
        sl = slice(h * NSH, (h + 1) * NSH)
        out[b, sl] = res[core]["out"]
        disp[b, sl] = res[core]["disp"].reshape(NSH, 1)
        conf[b, sl] = res[core]["conf"].reshape(NSH, 1)
        attn[b, sl] = res[core]["attn"]
    return (out, disp, conf, attn)


# revision 10
# speedup vs baseline: 1.0198x; 1.0198x over previous
"""Epipolar cross-attention Trainium2 kernel.

Full inputs -> shard over 8 cores as (batch b = core//2, query-half = core%2),
each core computes 2048 query rows against all M=4096 keys of its batch.

Math (matches reference exactly):
  Q = nodes_L @ Wq^T + bq ; K = nodes_R @ Wk^T + bk ; V = nodes_R @ Wv^T + bv
  mask = (|vL-vR| < 3) & (uL-uR > 0) & (uL-uR < 192)
  S = where(mask, QK^T/sqrt(C), -1e9); W = softmax(S)
  out = (W@V) @ Wm^T + bm ; disp = sum(W*(uL-uR)) ; conf = any(mask)

Device trick: penalty form. q = mask*512 (exact fp32 compares on DVE),
PSUM += identity @ q (bf16), then E = exp(S + 512*mask - 512):
  masked -> exp(s-512) == 0 exactly (fp32 underflow)  [matches where(): 0]
  valid  -> exp(s) (logit quantized at ulp(512)=6.1e-5)
Empty rows: E == 0 -> Z == 0 -> W = 1/4096 uniform via fused scalar add.
W @ [V, uR, 1] aug-matmul gives matched, sum(W*uR), sum(W) in one pass;
disp = uL*sumW - sum(W*uR) handles uniform rows automatically.
"""
import os
import numpy as np

B, N, M, C = 4, 4096, 4096, 256
NSH = N // 2          # rows per core
P = 128
NB = NSH // P         # 16 n-blocks
MC = 1024             # m-chunk
NMC = M // MC         # 4 chunks
PEN = 512.0           # penalty scale (exact in bf16; exp(s-512) == 0)

_CACHE = {}
LAST_RESULT = None


def _build():
    import concourse.bacc as bacc
    import concourse.tile as tile
    import concourse.mybir as mybir
    from concourse.masks import make_identity

    f32 = mybir.dt.float32
    f32r = mybir.dt.float32r
    bf16 = mybir.dt.bfloat16
    A = mybir.AluOpType
    AF = mybir.ActivationFunctionType

    nc = bacc.Bacc(None)

    nl_d = nc.dram_tensor("nodesL", [NSH, C], f32, kind="ExternalInput")
    nr_d = nc.dram_tensor("nodesR", [M, C], f32, kind="ExternalInput")
    uL_d = nc.dram_tensor("uL", [NSH], f32, kind="ExternalInput")
    vL_d = nc.dram_tensor("vL", [NSH], f32, kind="ExternalInput")
    uR_d = nc.dram_tensor("uR", [1, M], f32, kind="ExternalInput")
    vR_d = nc.dram_tensor("vR", [1, M], f32, kind="ExternalInput")
    wq_d = nc.dram_tensor("Wq", [C, C], f32, kind="ExternalInput")
    wk_d = nc.dram_tensor("Wk", [C, C], f32, kind="ExternalInput")
    wv_d = nc.dram_tensor("Wv", [C, C], f32, kind="ExternalInput")
    wm_d = nc.dram_tensor("Wm", [C, C], f32, kind="ExternalInput")
    bq_d = nc.dram_tensor("bq", [1, C], f32, kind="ExternalInput")
    bk_d = nc.dram_tensor("bk", [1, C], f32, kind="ExternalInput")
    bv_d = nc.dram_tensor("bv", [1, C], f32, kind="ExternalInput")
    bm_d = nc.dram_tensor("bm", [1, C], f32, kind="ExternalInput")

    attn_d = nc.dram_tensor("attn", [NSH, M], f32, kind="ExternalOutput")
    out_d = nc.dram_tensor("out", [NSH, C], f32, kind="ExternalOutput")
    disp_d = nc.dram_tensor("disp", [NSH], f32, kind="ExternalOutput")
    conf_d = nc.dram_tensor("conf", [NSH], f32, kind="ExternalOutput")

    KC = C // P  # 2 contraction chunks
    attn_v = attn_d.rearrange("(nb p) m -> nb p m", p=P)
    out_v = out_d.rearrange("(nb p) c -> nb p c", p=P)

    with tile.TileContext(nc) as tc:
        with tc.tile_pool(name="const", bufs=1) as cst:
            # ---------------- resident tensors ----------------
            qT = cst.tile([P, KC, NSH], f32r, tag="qT")        # [d, n]
            kT = cst.tile([P, KC, M], f32r, tag="kT")          # [d, m]
            vaug = cst.tile([P, M // P, C + 2], f32r, tag="vaug")  # [m, d|uR|1]
            uRb = cst.tile([P, M], f32, tag="uRb")
            vRb = cst.tile([P, M], f32, tag="vRb")
            uL_c = cst.tile([P, NB], f32, tag="uL_c")
            vL_c = cst.tile([P, NB], f32, tag="vL_c")
            idb = cst.tile([P, P], bf16, tag="idb")            # identity bf16
            idr = cst.tile([P, P], f32r, tag="idr")            # identity f32r
            ones512 = cst.tile([1, 512], f32r, tag="ones512")
            onesP = cst.tile([1, P], f32r, tag="onesP")
            bqr = cst.tile([1, C], f32r, tag="bqr")            # bq/16
            bkr = cst.tile([1, C], f32r, tag="bkr")
            bvr = cst.tile([1, C], f32r, tag="bvr")
            bmr = cst.tile([1, C], f32r, tag="bmr")
            wqT = cst.tile([P, KC, C], f32r, tag="wqT")        # [c_in, d]/16
            wkT = cst.tile([P, KC, C], f32r, tag="wkT")
            wvT = cst.tile([P, KC, C], f32r, tag="wvT")
            wmT = cst.tile([P, KC, C], f32r, tag="wmT")        # [d, c]
            nb512 = cst.tile([P, 1], f32, tag="nb512")         # -512 bias
            disp_all = cst.tile([P, NB], f32, tag="disp_all")
            conf_all = cst.tile([P, NB], f32, tag="conf_all")

            # ---------------- loads ----------------
            nc.sync.dma_start(uL_c[:], uL_d.rearrange("(nb p) -> p nb", p=P))
            nc.sync.dma_start(vL_c[:], vL_d.rearrange("(nb p) -> p nb", p=P))
            nc.sync.dma_start(uRb[:], uR_d[:].partition_broadcast(P))
            nc.sync.dma_start(vRb[:], vR_d[:].partition_broadcast(P))
            nc.gpsimd.dma_start(vaug[:, :, C],
                                uR_d.rearrange("one (a p) -> (one p) a", p=P))

            make_identity(nc, idb[:])
            idf = cst.tile([P, P], f32, tag="idf")
            make_identity(nc, idf[:])
            nc.scalar.copy(idr[:], idf[:])
            zsrc = cst.tile([1, 512], f32, tag="zsrc")
            nc.gpsimd.memset(zsrc[:], 0.0)
            nc.scalar.activation(ones512[:], zsrc[:], AF.Copy, bias=1.0, scale=0.0)
            nc.scalar.activation(onesP[:], zsrc[:, :P], AF.Copy, bias=1.0, scale=0.0)
            # vaug ones column: value-independent copy-const from uRb slice
            nc.scalar.activation(vaug[:, :, C + 1], uRb[:, : M // P],
                                 AF.Copy, bias=1.0, scale=0.0)
            nc.gpsimd.memset(nb512[:], -PEN)

            btmp = cst.tile([1, C], f32, tag="btmp")
            for b_dram, b_sb, scl in ((bq_d, bqr, 0.0625), (bk_d, bkr, 1.0),
                                      (bv_d, bvr, 1.0), (bm_d, bmr, 1.0)):
                nc.sync.dma_start(btmp[:], b_dram[:])
                nc.scalar.activation(b_sb[:], btmp[:], AF.Copy, bias=0.0, scale=scl)

            # ---------------- prologue: transposes + projections ----------------
            with (
                tc.tile_pool(name="prol", bufs=2) as prol,
                tc.tile_pool(name="ppsum", bufs=2, space="PSUM") as pps,
            ):
                # weights: natural [row-part, col] -> transposed [col-part, row]
                for w_dram, w_out, scl in (
                    (wq_d, wqT, 0.0625), (wk_d, wkT, 1.0), (wv_d, wvT, 1.0),
                    (wm_d, wmT, 1.0),
                ):
                    wnat = prol.tile([P, KC, C], f32r, tag="wnat")
                    nc.gpsimd.dma_start(wnat[:],
                                        w_dram.rearrange("(a p) c -> p a c", p=P))
                    for a in range(KC):
                        for bb in range(KC):
                            tp = pps.tile([P, P], f32r, tag="tp_w")
                            nc.tensor.transpose(tp[:], wnat[:, a, bb * P:(bb + 1) * P],
                                                idr[:])
                            nc.scalar.activation(w_out[:, bb, a * P:(a + 1) * P],
                                                 tp[:].bitcast(f32), AF.Copy,
                                                 bias=0.0, scale=scl)

                # nodes_R^T halves; K^T[d,m] (+bk); V[m,d] (+bv) -> vaug
                mh = M // 2
                for half in range(2):
                    nrT = prol.tile([P, KC, mh], f32r, tag="nrT")
                    nrnat = prol.tile([P, mh // P, C], f32r, tag="nrnat")
                    nc.gpsimd.dma_start(
                        nrnat[:],
                        nr_d[half * mh:(half + 1) * mh].rearrange("(a p) c -> p a c", p=P))
                    for a2 in range(mh // (2 * P)):
                        tpn = pps.tile([P, KC, 2 * P], f32r, tag="tp_n")
                        for a1 in range(2):
                            a = a2 * 2 + a1
                            for bb in range(KC):
                                nc.tensor.transpose(tpn[:, bb, a1 * P:(a1 + 1) * P],
                                                    nrnat[:, a, bb * P:(bb + 1) * P],
                                                    idr[:])
                        nc.scalar.copy(nrT[:, :, a2 * 2 * P:(a2 + 1) * 2 * P],
                                       tpn[:].bitcast(f32))
                    for dd in range(KC):
                        for mt in range(mh // 512):
                            ps_k = pps.tile([P, 512], f32, tag="ps_k")
                            for k in range(KC):
                                nc.tensor.matmul(ps_k[:], wkT[:, k, dd * P:(dd + 1) * P],
                                                 nrT[:, k, mt * 512:(mt + 1) * 512],
                                                 start=(k == 0), stop=False)
                            nc.tensor.matmul(ps_k[:], bkr[:, dd * P:(dd + 1) * P],
                                             ones512[:], start=False, stop=True)
                            lo = half * mh + mt * 512
                            nc.vector.tensor_copy(kT[:, dd, lo:lo + 512], ps_k[:])
                    for a in range(mh // P):
                        ps_v = pps.tile([P, C], f32, tag="ps_v")
                        for k in range(KC):
                            nc.tensor.matmul(ps_v[:], nrT[:, k, a * P:(a + 1) * P],
                                             wvT[:, k, :], start=(k == 0), stop=False)
                        nc.tensor.matmul(ps_v[:], onesP[:], bvr[:],
                                         start=False, stop=True)
                        nc.vector.tensor_copy(vaug[:, half * (mh // P) + a, :C], ps_v[:])

                # nodes_L^T halves; Q^T[d,n] = (Wq/16)@nlT + bq/16
                nh = NSH // 2
                for half in range(2):
                    nlT = prol.tile([P, KC, nh], f32r, tag="nrT")
                    nlnat = prol.tile([P, nh // P, C], f32r, tag="nrnat")
                    nc.gpsimd.dma_start(
                        nlnat[:],
                        nl_d[half * nh:(half + 1) * nh].rearrange("(a p) c -> p a c", p=P))
                    for a2 in range(nh // (2 * P)):
                        tpn = pps.tile([P, KC, 2 * P], f32r, tag="tp_n")
                        for a1 in range(2):
                            a = a2 * 2 + a1
                            for bb in range(KC):
                                nc.tensor.transpose(tpn[:, bb, a1 * P:(a1 + 1) * P],
                                                    nlnat[:, a, bb * P:(bb + 1) * P],
                                                    idr[:])
                        nc.scalar.copy(nlT[:, :, a2 * 2 * P:(a2 + 1) * 2 * P],
                                       tpn[:].bitcast(f32))
                    for dd in range(KC):
                        for nt in range(nh // 512):
                            ps_q = pps.tile([P, 512], f32, tag="ps_k")
                            for k in range(KC):
                                nc.tensor.matmul(ps_q[:], wqT[:, k, dd * P:(dd + 1) * P],
                                                 nlT[:, k, nt * 512:(nt + 1) * 512],
                                                 start=(k == 0), stop=False)
                            nc.tensor.matmul(ps_q[:], bqr[:, dd * P:(dd + 1) * P],
                                             ones512[:], start=False, stop=True)
                            lo = half * nh + nt * 512
                            nc.scalar.copy(qT[:, dd, lo:lo + 512], ps_q[:])

            # ---------------- main loop ----------------
            with (
                tc.tile_pool(name="mn", bufs=2) as mn,
                tc.tile_pool(name="epool", bufs=5) as epool,
                tc.tile_pool(name="wpool", bufs=3) as wpool,
                tc.tile_pool(name="small", bufs=2) as sml,
                tc.tile_pool(name="wtp", bufs=4) as wtp,
                tc.tile_pool(name="qk_ps", bufs=2, space="PSUM") as qk_ps,
                tc.tile_pool(name="tp_ps", bufs=2, space="PSUM") as tp_ps,
                tc.tile_pool(name="mt_ps", bufs=1, space="PSUM") as mt_ps,
                tc.tile_pool(name="op_ps", bufs=1, space="PSUM") as op_ps,
            ):
                for i in range(NB):
                    uL_i = uL_c[:, i:i + 1]
                    vL_i = vL_c[:, i:i + 1]
                    z4 = sml.tile([P, NMC], f32, tag="z4")
                    e_ch = []
                    for h in range(NMC):
                        ms = h * MC
                        dv = mn.tile([P, MC], f32, tag="dv")
                        nc.scalar.activation(dv[:], vRb[:, ms:ms + MC], AF.Abs,
                                             bias=vL_i, scale=-1.0)
                        cv = mn.tile([P, MC], bf16, tag="cv")
                        nc.vector.tensor_scalar(cv[:], dv[:], 3.0, PEN,
                                                A.is_lt, A.mult)
                        cu2 = mn.tile([P, MC], bf16, tag="cu2")
                        nc.vector.tensor_scalar(cu2[:], uRb[:, ms:ms + MC], uL_i,
                                                -192.0, A.subtract, A.is_gt)
                        t1 = mn.tile([P, MC], bf16, tag="t1")
                        nc.vector.scalar_tensor_tensor(t1[:], uRb[:, ms:ms + MC],
                                                       uL_i, cv[:], A.is_lt, A.mult)
                        qm = mn.tile([P, MC], bf16, tag="qm")
                        nc.vector.tensor_tensor(qm[:], t1[:], cu2[:], A.mult)
                        ps = qk_ps.tile([P, MC], f32, tag="ps_qk")
                        for t in range(MC // 512):
                            sl = slice(t * 512, (t + 1) * 512)
                            for k in range(KC):
                                nc.tensor.matmul(ps[:, sl],
                                                 qT[:, k, i * P:(i + 1) * P],
                                                 kT[:, k, ms + t * 512:ms + (t + 1) * 512],
                                                 start=(k == 0), stop=False)
                            nc.tensor.matmul(ps[:, sl], idb[:], qm[:, sl],
                                             start=False, stop=True)
                        e = epool.tile([P, MC], f32, tag="e")
                        nc.scalar.activation(e[:], ps[:], AF.Exp, bias=nb512[:],
                                             scale=1.0, accum_out=z4[:, h:h + 1])
                        e_ch.append(e)

                    z = sml.tile([P, 1], f32, tag="z")
                    nc.vector.tensor_reduce(z[:], z4[:], mybir.AxisListType.X, A.add)
                    zadj = sml.tile([P, 1], f32, tag="zadj")
                    nc.vector.scalar_tensor_tensor(zadj[:], z[:], 0.0, z[:],
                                                   A.is_equal, A.add)
                    rz = sml.tile([P, 1], f32, tag="rz")
                    nc.vector.reciprocal(rz[:], zadj[:])
                    ucol = sml.tile([P, 1], f32, tag="ucol")
                    nc.vector.tensor_scalar(ucol[:], z[:], 0.0, 1.0 / M,
                                            A.is_equal, A.mult)
                    nc.vector.tensor_scalar(conf_all[:, i:i + 1], z[:], 0.0, None,
                                            A.is_gt)

                    mt_acc = mt_ps.tile([P, C + 2], f32, tag="mt_acc")
                    for h in range(NMC):
                        w = wpool.tile([P, MC], f32r, tag="w")
                        nc.vector.tensor_scalar(w[:], e_ch[h][:], rz[:], ucol[:],
                                                A.mult, A.add)
                        nc.sync.dma_start(attn_v[i, :, h * MC:(h + 1) * MC],
                                          w[:].bitcast(f32))
                        # 4 transposes per PSUM group, one batched copy, 2 groups
                        for g in range(2):
                            tp = tp_ps.tile([P, MC // 2], f32r, tag="tp_main")
                            for j in range(MC // (2 * P)):
                                jj = g * (MC // (2 * P)) + j
                                nc.tensor.transpose(tp[:, j * P:(j + 1) * P],
                                                    w[:, jj * P:(jj + 1) * P], idr[:])
                            wT = wtp.tile([P, MC // 2], f32r, tag="wT")
                            if (2 * h + g) % 3 == 2:
                                nc.vector.tensor_copy(wT[:], tp[:].bitcast(f32))
                            else:
                                nc.scalar.copy(wT[:], tp[:].bitcast(f32))
                            for j in range(MC // (2 * P)):
                                jg = h * (MC // P) + g * (MC // (2 * P)) + j
                                nc.tensor.matmul(mt_acc[:], wT[:, j * P:(j + 1) * P],
                                                 vaug[:, jg, :],
                                                 start=(jg == 0),
                                                 stop=(jg == M // P - 1))

                    mt_sb = sml.tile([P, C + 2], f32r, tag="mt_sb")
                    nc.scalar.copy(mt_sb[:], mt_acc[:])
                    nc.vector.scalar_tensor_tensor(
                        disp_all[:, i:i + 1], mt_sb[:, C + 1:C + 2].bitcast(f32),
                        uL_i, mt_sb[:, C:C + 1].bitcast(f32), A.mult, A.subtract)

                    mTt = sml.tile([P, KC, P], f32r, tag="mTt")
                    tpm = tp_ps.tile([P, MC // 2], f32r, tag="tp_main")
                    for k in range(KC):
                        nc.tensor.transpose(tpm[:, k * P:(k + 1) * P],
                                            mt_sb[:, k * P:(k + 1) * P], idr[:])
                    nc.scalar.copy(mTt[:], tpm[:, :KC * P].bitcast(f32))
                    op = op_ps.tile([P, C], f32, tag="op")
                    for k in range(KC):
                        nc.tensor.matmul(op[:], mTt[:, k, :], wmT[:, k, :],
                                         start=(k == 0), stop=False)
                    nc.tensor.matmul(op[:], onesP[:], bmr[:], start=False, stop=True)
                    out_sb = sml.tile([P, C], f32, tag="out_sb")
                    nc.scalar.copy(out_sb[:], op[:])
                    nc.sync.dma_start(out_v[i], out_sb[:])

                nc.sync.dma_start(disp_d.rearrange("(nb p) -> p nb", p=P), disp_all[:])
                nc.sync.dma_start(conf_d.rearrange("(nb p) -> p nb", p=P), conf_all[:])

    nc.compile()
    return nc


def kernel(nodes_L, nodes_R, kpts_L, kpts_R, Wq, bq, Wk, bk, Wv, bv, Wm, bm):
    global LAST_RESULT
    from concourse.bass_utils import run_bass_kernel_spmd

    if "nc" not in _CACHE:
        _CACHE["nc"] = _build()
    nc = _CACHE["nc"]

    nodes_L = np.ascontiguousarray(np.asarray(nodes_L, dtype=np.float32))
    nodes_R = np.ascontiguousarray(np.asarray(nodes_R, dtype=np.float32))
    kpts_L = np.asarray(kpts_L, dtype=np.float32)
    kpts_R = np.asarray(kpts_R, dtype=np.float32)
    common = {
        "Wq": np.ascontiguousarray(np.asarray(Wq, np.float32)),
        "Wk": np.ascontiguousarray(np.asarray(Wk, np.float32)),
        "Wv": np.ascontiguousarray(np.asarray(Wv, np.float32)),
        "Wm": np.ascontiguousarray(np.asarray(Wm, np.float32)),
        "bq": np.ascontiguousarray(np.asarray(bq, np.float32)).reshape(1, C),
        "bk": np.ascontiguousarray(np.asarray(bk, np.float32)).reshape(1, C),
        "bv": np.ascontiguousarray(np.asarray(bv, np.float32)).reshape(1, C),
        "bm": np.ascontiguousarray(np.asarray(bm, np.float32)).reshape(1, C),
    }
    in_maps = []
    for core in range(8):
        b, h = core // 2, core % 2
        sl = slice(h * NSH, (h + 1) * NSH)
        in_maps.append({
            "nodesL": np.ascontiguousarray(nodes_L[b, sl]),
            "nodesR": np.ascontiguousarray(nodes_R[b]),
            "uL": np.ascontiguousarray(kpts_L[b, sl, 0]),
            "vL": np.ascontiguousarray(kpts_L[b, sl, 1]),
            "uR": np.ascontiguousarray(kpts_R[b, :, 0]).reshape(1, M),
            "vR": np.ascontiguousarray(kpts_R[b, :, 1]).reshape(1, M),
            **common,
        })

    LAST_RESULT = run_bass_kernel_spmd(nc, in_maps, core_ids=list(range(8)))
    res = LAST_RESULT.results

    out = np.empty((B, N, C), np.float32)
    disp = np.empty((B, N, 1), np.float32)
    conf = np.empty((B, N, 1), np.float32)
    attn = np.empty((B, N, M), np.float32)
    for core in range(8):
        b, h = core // 2, core % 2
        sl = slice(h * NSH, (h + 1) * NSH)
        out[b, sl] = res[core]["out"]
        disp[b, sl] = res[core]["disp"].reshape(NSH, 1)
        conf[b, sl] = res[core]["conf"].reshape(NSH, 1)
        attn[b, sl] = res[core]["attn"]
    return (out, disp, conf, attn)
